# revision 2
# baseline (speedup 1.0000x reference)
"""Trainium2 Bass kernel for a detection-head MLP (conv7x7-as-matmul ->
dense -> {classifier softmax, box offsets}), data-parallel over the ROI
axis N on 8 NeuronCores.

Self-contained: hardcodes shapes from the problem spec.
  x  [4096,7,7,256] -> flatten [4096,12544]
  y1 = relu(bn(x @ W1))          [4096,1024]
  y2 = relu(bn(y1 @ W2))         [4096,1024]
  logit = y2 @ wc + bc           [4096,81]
  prob  = softmax(logit)
  off   = (y2 @ wo + bo)         [4096,81,4]

Per-core layout (512 ROIs/core): activations kept transposed
[channels(partition), rows(free)] so every layer is a chain of
128x128-stationary x [128,512]-moving matmuls accumulating in PSUM.
Final layer flips: stationary = y2T tile (k x rows), moving = [wc|wo]
(k x 405), giving row-major [128 rows, 405] outputs; softmax runs along
the free dim. BN+bias folded into a per-channel affine applied during
PSUM->SBUF evacuation on the scalar engine.
"""

import sys
import types

sys.path.insert(0, "/opt/trn_rl_repo")

import numpy as np
import ml_dtypes
from contextlib import ExitStack

import concourse.bass as bass
import concourse.tile as tile
from concourse import bacc, mybir
from concourse.bass_utils import run_bass_kernel_spmd

# ---- problem constants ----
N_FULL = 4096
K1 = 7 * 7 * 256  # 12544
H = 1024
NCLS = 81
NOFF = NCLS * 4  # 324
NCO = NCLS + NOFF  # 405
EPS = 1e-3
NCORES = 8
NS = N_FULL // NCORES  # 512 rows per core
KT1 = K1 // 128  # 98
KT2 = H // 128  # 8
RT = NS // 128  # 4 row-tiles per core

# matmul operand dtype: bfloat16 (fast, ~5e-3 rel err) or float32r
# (TF32-like, ~5e-4 rel err, 2x DMA bytes). Accumulation is fp32 either way.
MM_DTYPE = "bfloat16"

_DT_MAP = {
    "bfloat16": (mybir.dt.bfloat16, ml_dtypes.bfloat16),
    "float32r": (mybir.dt.float32r, np.float32),
}


def _install_ntff_hook():
    """Best-effort registration of the axon NTFF profile hook so callers
    can request trace=True. No-op if the plumbing is unavailable."""
    try:
        import antenv

        if "antenv.axon_hooks" not in sys.modules:
            hooks = types.ModuleType("antenv.axon_hooks")
            hooks._hook = None
            hooks.set_axon_ntff_profile_hook = lambda h: setattr(hooks, "_hook", h)
            hooks.get_axon_ntff_profile_hook = lambda: hooks._hook
            sys.modules["antenv.axon_hooks"] = hooks
            antenv.axon_hooks = hooks
            from trn_agent_boot.trn_boot import _ntff_profile_via_ctypes

            hooks.set_axon_ntff_profile_hook(
                _ntff_profile_via_ctypes("/opt/axon/libaxon_pjrt.so")
            )
    except Exception:
        pass


_PROG = None


def _build_program(mm_dtype_name=MM_DTYPE):
    dt_mm, _ = _DT_MAP[mm_dtype_name]
    f32 = mybir.dt.float32

    nc = bacc.Bacc("TRN2", target_bir_lowering=False, debug=False)

    xT_d = nc.dram_tensor("xT", [K1, NS], dt_mm, kind="ExternalInput").ap()
    w1_d = nc.dram_tensor("w1", [K1, H], dt_mm, kind="ExternalInput").ap()
    w2_d = nc.dram_tensor("w2", [H, H], dt_mm, kind="ExternalInput").ap()
    wco_d = nc.dram_tensor("wco", [128, KT2 * NCO], dt_mm, kind="ExternalInput").ap()
    bco_d = nc.dram_tensor("bco", [1, NCO], dt_mm, kind="ExternalInput").ap()
    sb1_d = nc.dram_tensor("sb1", [H, 2], f32, kind="ExternalInput").ap()
    sb2_d = nc.dram_tensor("sb2", [H, 2], f32, kind="ExternalInput").ap()

    lg_d = nc.dram_tensor("logits", [NS, NCLS], f32, kind="ExternalOutput").ap()
    pr_d = nc.dram_tensor("probs", [NS, NCLS], f32, kind="ExternalOutput").ap()
    of_d = nc.dram_tensor("offs", [NS, NOFF], f32, kind="ExternalOutput").ap()

    with tile.TileContext(nc) as tc:
        with ExitStack() as ctx:
            const = ctx.enter_context(tc.tile_pool(name="const", bufs=1))
            xpool = ctx.enter_context(tc.tile_pool(name="x", bufs=4))
            w1pool = ctx.enter_context(tc.tile_pool(name="w1", bufs=4))
            w2pool = ctx.enter_context(tc.tile_pool(name="w2", bufs=2))
            ypool = ctx.enter_context(tc.tile_pool(name="y", bufs=1))
            opool = ctx.enter_context(tc.tile_pool(name="o", bufs=2))
            pspool = ctx.enter_context(tc.tile_pool(name="ps", bufs=1, space="PSUM"))

            # resident constants
            wco_sb = const.tile([128, KT2 * NCO], dt_mm, tag="wco")
            nc.sync.dma_start(wco_sb[:], wco_d[:])
            bco_sb = const.tile([1, NCO], dt_mm, tag="bco")
            nc.sync.dma_start(bco_sb[:], bco_d[:])
            ones_sb = const.tile([1, 128], dt_mm, tag="ones")
            nc.vector.memset(ones_sb[:], 1.0)
            sb1_t, sb2_t = [], []
            for m in range(KT2):
                t1 = const.tile([128, 2], f32, tag=f"sb1_{m}", name=f"sb1t_{m}")
                nc.sync.dma_start(t1[:], sb1_d[m * 128 : (m + 1) * 128, :])
                sb1_t.append(t1)
                t2 = const.tile([128, 2], f32, tag=f"sb2_{m}", name=f"sb2t_{m}")
                nc.sync.dma_start(t2[:], sb2_d[m * 128 : (m + 1) * 128, :])
                sb2_t.append(t2)

            # ---- phase 1: y1T[m] = relu(affine(sum_k W1[k,m].T @ xT[k])) ----
            ps1 = [pspool.tile([128, NS], f32, tag=f"ps_{m}", name=f"ps1_{m}") for m in range(KT2)]
            for k in range(KT1):
                xt = xpool.tile([128, NS], dt_mm, tag="x")
                nc.sync.dma_start(xt[:], xT_d[k * 128 : (k + 1) * 128, :])
                w1t = w1pool.tile([128, H], dt_mm, tag="w1")
                nc.sync.dma_start(w1t[:], w1_d[k * 128 : (k + 1) * 128, :])
                for m in range(KT2):
                    nc.tensor.matmul(
                        ps1[m][:],
                        w1t[:, m * 128 : (m + 1) * 128],
                        xt[:],
                        start=(k == 0),
                        stop=(k == KT1 - 1),
                    )
            y1 = []
            for m in range(KT2):
                ym = ypool.tile([128, NS], dt_mm, tag=f"y1_{m}", name=f"y1t_{m}")
                nc.scalar.activation(
                    ym[:],
                    ps1[m][:],
                    mybir.ActivationFunctionType.Relu,
                    bias=sb1_t[m][:, 1:2],
                    scale=sb1_t[m][:, 0:1],
                )
                y1.append(ym)

            # ---- phase 2: y2T[m] = relu(affine(sum_k W2[k,m].T @ y1T[k])) ----
            ps2 = [pspool.tile([128, NS], f32, tag=f"ps_{m}", name=f"ps2_{m}") for m in range(KT2)]
            for k in range(KT2):
                w2t = w2pool.tile([128, H], dt_mm, tag="w2")
                nc.sync.dma_start(w2t[:], w2_d[k * 128 : (k + 1) * 128, :])
                for m in range(KT2):
                    nc.tensor.matmul(
                        ps2[m][:],
                        w2t[:, m * 128 : (m + 1) * 128],
                        y1[k][:],
                        start=(k == 0),
                        stop=(k == KT2 - 1),
                    )
            y2 = []
            for m in range(KT2):
                ym = ypool.tile([128, NS], dt_mm, tag=f"y2_{m}", name=f"y2t_{m}")
                nc.scalar.activation(
                    ym[:],
                    ps2[m][:],
                    mybir.ActivationFunctionType.Relu,
                    bias=sb2_t[m][:, 1:2],
                    scale=sb2_t[m][:, 0:1],
                )
                y2.append(ym)

            # ---- phase 3: per 128-row tile, [rows, 405] = y2T.T @ [wc|wo] ----
            for r in range(RT):
                ps3 = pspool.tile([128, NCO], f32, tag=f"ps_{r}", name=f"ps3_{r}")
                # bias row: ones[1,128].T @ bco[1,405] broadcasts bco to all rows
                nc.tensor.matmul(ps3[:], ones_sb[:], bco_sb[:], start=True, stop=False)
                for k in range(KT2):
                    nc.tensor.matmul(
                        ps3[:],
                        y2[k][:, r * 128 : (r + 1) * 128],
                        wco_sb[:, k * NCO : (k + 1) * NCO],
                        start=False,
                        stop=(k == KT2 - 1),
                    )
                row = slice(r * 128, (r + 1) * 128)
                lg = opool.tile([128, NCLS], f32, tag="lg")
                nc.vector.tensor_copy(lg[:], ps3[:, 0:NCLS])
                nc.sync.dma_start(lg_d[row, :], lg[:])
                off = opool.tile([128, NOFF], f32, tag="off")
                nc.vector.tensor_copy(off[:], ps3[:, NCLS:NCO])
                nc.sync.dma_start(of_d[row, :], off[:])
                negmax = opool.tile([128, 1], f32, tag="nm")
                nc.vector.tensor_reduce(
                    negmax[:],
                    ps3[:, 0:NCLS],
                    mybir.AxisListType.X,
                    mybir.AluOpType.max,
                    negate=True,
                )
                expt = opool.tile([128, NCLS], f32, tag="ex")
                sumexp = opool.tile([128, 1], f32, tag="se")
                nc.scalar.activation(
                    expt[:],
                    ps3[:, 0:NCLS],
                    mybir.ActivationFunctionType.Exp,
                    bias=negmax[:, 0:1],
                    scale=1.0,
                    accum_out=sumexp[:, 0:1],
                )
                recip = opool.tile([128, 1], f32, tag="rc")
                nc.vector.reciprocal(recip[:], sumexp[:])
                prob = opool.tile([128, NCLS], f32, tag="pr")
                nc.vector.tensor_scalar_mul(prob[:], expt[:], recip[:, 0:1])
                nc.sync.dma_start(pr_d[row, :], prob[:])

    nc.compile()
    return nc


def _prep_inputs(inputs, mm_dtype_name=MM_DTYPE):
    """Host-side reshape/fold/cast. Returns per-core input maps."""
    _, np_mm = _DT_MAP[mm_dtype_name]
    f32 = np.float32

    x = np.asarray(inputs["x"], f32).reshape(N_FULL, K1)
    W1 = np.asarray(inputs["w1"], f32).reshape(K1, H)
    W2 = np.asarray(inputs["w2"], f32)
    wc, bc = np.asarray(inputs["wc"], f32), np.asarray(inputs["bc"], f32)
    wo, bo = np.asarray(inputs["wo"], f32), np.asarray(inputs["bo"], f32)

    s1 = inputs["g1"] / np.sqrt(inputs["v1"] + EPS)
    b1f = (inputs["b1"] - inputs["m1"]) * s1 + inputs["be1"]
    s2 = inputs["g2"] / np.sqrt(inputs["v2"] + EPS)
    b2f = (inputs["b2"] - inputs["m2"]) * s2 + inputs["be2"]
    sb1 = np.stack([s1, b1f], axis=1).astype(f32)  # [1024, 2]
    sb2 = np.stack([s2, b2f], axis=1).astype(f32)

    wco = np.concatenate([wc, wo], axis=1)  # [1024, 405]
    # pack k-tiles along free dim: wcoP[p, k*405+c] = wco[k*128+p, c]
    wcoP = np.ascontiguousarray(
        wco.reshape(KT2, 128, NCO).transpose(1, 0, 2).reshape(128, KT2 * NCO)
    ).astype(np_mm)
    bco = np.concatenate([bc, bo]).reshape(1, NCO).astype(np_mm)

    w1c = np.ascontiguousarray(W1).astype(np_mm)
    w2c = np.ascontiguousarray(W2).astype(np_mm)

    in_maps = []
    for c in range(NCORES):
        xT = np.ascontiguousarray(x[c * NS : (c + 1) * NS].T).astype(np_mm)
        in_maps.append(
            {
                "xT": xT,
                "w1": w1c,
                "w2": w2c,
                "wco": wcoP,
                "bco": bco,
                "sb1": sb1,
                "sb2": sb2,
            }
        )
    return in_maps


def _gather(results):
    logits = np.concatenate([results[c]["logits"] for c in range(NCORES)], axis=0)
    probs = np.concatenate([results[c]["probs"] for c in range(NCORES)], axis=0)
    offs = np.concatenate([results[c]["offs"] for c in range(NCORES)], axis=0)
    return (
        logits.astype(np.float32),
        probs.astype(np.float32),
        offs.reshape(N_FULL, NCLS, 4).astype(np.float32),
    )


def run(inputs, trace=False):
    """Run on 8 cores. Returns ((logits, probs, offset), exec_time_ns)."""
    global _PROG
    if trace:
        _install_ntff_hook()
    if _PROG is None:
        _PROG = _build_program()
    in_maps = _prep_inputs(inputs)
    res = run_bass_kernel_spmd(_PROG, in_maps, list(range(NCORES)), trace=trace)
    return _gather(res.results), res.exec_time_ns


def kernel(**inputs):
    out, _ = run(inputs, trace=False)
    return out


# revision 3
# speedup vs baseline: 1.0610x; 1.0610x over previous
"""Trainium2 Bass kernel for a detection-head MLP (conv7x7-as-matmul ->
dense -> {classifier softmax, box offsets}), data-parallel over the ROI
axis N on 8 NeuronCores.

Self-contained: hardcodes shapes from the problem spec.
  x  [4096,7,7,256] -> flatten [4096,12544]
  y1 = relu(bn(x @ W1))          [4096,1024]
  y2 = relu(bn(y1 @ W2))         [4096,1024]
  logit = y2 @ wc + bc           [4096,81]
  prob  = softmax(logit)
  off   = (y2 @ wo + bo)         [4096,81,4]

Per-core layout (512 ROIs/core): activations kept transposed
[channels(partition), rows(free)] so every layer is a chain of
128x128-stationary x [128,512]-moving matmuls accumulating in PSUM.
Final layer flips: stationary = y2T tile (k x rows), moving = [wc|wo]
(k x 405), giving row-major [128 rows, 405] outputs; softmax runs along
the free dim. BN+bias folded into a per-channel affine applied during
PSUM->SBUF evacuation on the scalar engine.

All streamed inputs are host-packed so each DMA moves multiple k-tiles
with >=4KB contiguous per partition (small-packet DMA overhead dominated
the v1 profile). Group sizes ramp 1,2,4,7... so the first matmul starts
~1us after launch while steady-state transfers stay large.
"""

import sys
import types

sys.path.insert(0, "/opt/trn_rl_repo")

import numpy as np
import ml_dtypes
from contextlib import ExitStack

import concourse.bass as bass
import concourse.tile as tile
from concourse import bacc, mybir
from concourse.bass_utils import run_bass_kernel_spmd

# ---- problem constants ----
N_FULL = 4096
K1 = 7 * 7 * 256  # 12544
H = 1024
NCLS = 81
NOFF = NCLS * 4  # 324
NCO = NCLS + NOFF  # 405
NOUT = NCLS + NCO  # 486 = logits | probs | offsets packed
EPS = 1e-3
NCORES = 8
NS = N_FULL // NCORES  # 512 rows per core
KT1 = K1 // 128  # 98
KT2 = H // 128  # 8
RT = NS // 128  # 4 row-tiles per core

# k-tile group sizes for the streamed layer-1 operands (sum == KT1).
# Ramp-up start so the PE can begin within ~1us of launch.
GROUPS = [1, 2, 4] + [7] * 13
assert sum(GROUPS) == KT1

# matmul operand dtype: bfloat16 (fast, ~5e-3 rel err) or float32r
# (TF32-like, ~5e-4 rel err, 2x DMA bytes). Accumulation is fp32 either way.
MM_DTYPE = "bfloat16"

_DT_MAP = {
    "bfloat16": (mybir.dt.bfloat16, ml_dtypes.bfloat16),
    "float32r": (mybir.dt.float32r, np.float32),
}


def _install_ntff_hook():
    """Best-effort registration of the axon NTFF profile hook so callers
    can request trace=True. No-op if the plumbing is unavailable."""
    try:
        import antenv

        if "antenv.axon_hooks" not in sys.modules:
            hooks = types.ModuleType("antenv.axon_hooks")
            hooks._hook = None
            hooks.set_axon_ntff_profile_hook = lambda h: setattr(hooks, "_hook", h)
            hooks.get_axon_ntff_profile_hook = lambda: hooks._hook
            sys.modules["antenv.axon_hooks"] = hooks
            antenv.axon_hooks = hooks
            from trn_agent_boot.trn_boot import _ntff_profile_via_ctypes

            hooks.set_axon_ntff_profile_hook(
                _ntff_profile_via_ctypes("/opt/axon/libaxon_pjrt.so")
            )
    except Exception:
        pass


_PROG = None


def _build_program(mm_dtype_name=MM_DTYPE):
    dt_mm, _ = _DT_MAP[mm_dtype_name]
    f32 = mybir.dt.float32

    nc = bacc.Bacc("TRN2", target_bir_lowering=False, debug=False)

    # xp[g]: k-tiles of xT packed along free dim    [128, gsz*512]
    # w1p[g]: k-tiles of W1 packed along free dim   [128, gsz*1024]
    xp_d = nc.dram_tensor("xp", [128, KT1 * NS], dt_mm, kind="ExternalInput").ap()
    w1_d = nc.dram_tensor("w1p", [128, KT1 * H], dt_mm, kind="ExternalInput").ap()
    w2_d = nc.dram_tensor("w2p", [128, KT2 * H], dt_mm, kind="ExternalInput").ap()
    wco_d = nc.dram_tensor("wcop", [128, KT2 * NCO], dt_mm, kind="ExternalInput").ap()
    bco_d = nc.dram_tensor("bco", [1, NCO], dt_mm, kind="ExternalInput").ap()
    # per-channel affine consts: [128, m*4 + (s1,b1,s2,b2)]
    sb_d = nc.dram_tensor("sb", [128, KT2 * 4], f32, kind="ExternalInput").ap()

    out_d = nc.dram_tensor("out", [NS, NOUT], f32, kind="ExternalOutput").ap()

    with tile.TileContext(nc) as tc:
        with ExitStack() as ctx:
            const = ctx.enter_context(tc.tile_pool(name="const", bufs=1))
            xpool = ctx.enter_context(tc.tile_pool(name="x", bufs=3))
            w1pool = ctx.enter_context(tc.tile_pool(name="w1", bufs=3))
            ypool = ctx.enter_context(tc.tile_pool(name="y", bufs=1))
            opool = ctx.enter_context(tc.tile_pool(name="o", bufs=2))
            pspool = ctx.enter_context(tc.tile_pool(name="ps", bufs=1, space="PSUM"))

            # resident constants
            sb_t = const.tile([128, KT2 * 4], f32, tag="sb")
            nc.sync.dma_start(sb_t[:], sb_d[:])
            wco_sb = const.tile([128, KT2 * NCO], dt_mm, tag="wco")
            nc.sync.dma_start(wco_sb[:], wco_d[:])
            bco_sb = const.tile([1, NCO], dt_mm, tag="bco")
            nc.sync.dma_start(bco_sb[:], bco_d[:])
            ones_sb = const.tile([1, 128], dt_mm, tag="ones")
            nc.vector.memset(ones_sb[:], 1.0)
            w2_sb = const.tile([128, KT2 * H], dt_mm, tag="w2")
            nc.sync.dma_start(w2_sb[:], w2_d[:])

            # ---- phase 1: y1T[m] = relu(affine(sum_k W1[k,m].T @ xT[k])) ----
            ps1 = [
                pspool.tile([128, NS], f32, tag=f"ps_{m}", name=f"ps1_{m}")
                for m in range(KT2)
            ]
            k0 = 0
            for g, gsz in enumerate(GROUPS):
                xt = xpool.tile([128, gsz * NS], dt_mm, tag="x", name=f"xt_{g}")
                nc.sync.dma_start(
                    xt[:], xp_d[:, k0 * NS : (k0 + gsz) * NS]
                )
                w1t = w1pool.tile([128, gsz * H], dt_mm, tag="w1", name=f"w1t_{g}")
                nc.sync.dma_start(w1t[:], w1_d[:, k0 * H : (k0 + gsz) * H])
                for j in range(gsz):
                    k = k0 + j
                    for m in range(KT2):
                        nc.tensor.matmul(
                            ps1[m][:],
                            w1t[:, j * H + m * 128 : j * H + (m + 1) * 128],
                            xt[:, j * NS : (j + 1) * NS],
                            start=(k == 0),
                            stop=(k == KT1 - 1),
                        )
                k0 += gsz
            y1 = []
            for m in range(KT2):
                ym = ypool.tile([128, NS], dt_mm, tag=f"y1_{m}", name=f"y1t_{m}")
                nc.scalar.activation(
                    ym[:],
                    ps1[m][:],
                    mybir.ActivationFunctionType.Relu,
                    bias=sb_t[:, m * 4 + 1 : m * 4 + 2],
                    scale=sb_t[:, m * 4 : m * 4 + 1],
                )
                y1.append(ym)

            # ---- phase 2: y2T[m] = relu(affine(sum_k W2[k,m].T @ y1T[k])) ----
            ps2 = [
                pspool.tile([128, NS], f32, tag=f"ps_{m}", name=f"ps2_{m}")
                for m in range(KT2)
            ]
            for k in range(KT2):
                for m in range(KT2):
                    nc.tensor.matmul(
                        ps2[m][:],
                        w2_sb[:, k * H + m * 128 : k * H + (m + 1) * 128],
                        y1[k][:],
                        start=(k == 0),
                        stop=(k == KT2 - 1),
                    )
            y2 = []
            for m in range(KT2):
                ym = ypool.tile([128, NS], dt_mm, tag=f"y2_{m}", name=f"y2t_{m}")
                nc.scalar.activation(
                    ym[:],
                    ps2[m][:],
                    mybir.ActivationFunctionType.Relu,
                    bias=sb_t[:, m * 4 + 3 : m * 4 + 4],
                    scale=sb_t[:, m * 4 + 2 : m * 4 + 3],
                )
                y2.append(ym)

            # ---- phase 3: per 128-row tile, [rows, 405] = y2T.T @ [wc|wo] ----
            # out columns: [0:81) logits, [81:162) softmax probs, [162:486) offsets
            for r in range(RT):
                ps3 = pspool.tile([128, NCO], f32, tag=f"ps_{r}", name=f"ps3_{r}")
                # bias row: ones[1,128].T @ bco[1,405] broadcasts bco to all rows
                nc.tensor.matmul(ps3[:], ones_sb[:], bco_sb[:], start=True, stop=False)
                for k in range(KT2):
                    nc.tensor.matmul(
                        ps3[:],
                        y2[k][:, r * 128 : (r + 1) * 128],
                        wco_sb[:, k * NCO : (k + 1) * NCO],
                        start=False,
                        stop=(k == KT2 - 1),
                    )
                ot = opool.tile([128, NOUT], f32, tag="ot", name=f"ot_{r}")
                nc.vector.tensor_copy(ot[:, 0:NCLS], ps3[:, 0:NCLS])
                nc.vector.tensor_copy(ot[:, 2 * NCLS : NOUT], ps3[:, NCLS:NCO])
                negmax = opool.tile([128, 1], f32, tag="nm", name=f"nm_{r}")
                nc.vector.tensor_reduce(
                    negmax[:],
                    ps3[:, 0:NCLS],
                    mybir.AxisListType.X,
                    mybir.AluOpType.max,
                    negate=True,
                )
                sumexp = opool.tile([128, 1], f32, tag="se", name=f"se_{r}")
                nc.scalar.activation(
                    ot[:, NCLS : 2 * NCLS],
                    ps3[:, 0:NCLS],
                    mybir.ActivationFunctionType.Exp,
                    bias=negmax[:, 0:1],
                    scale=1.0,
                    accum_out=sumexp[:, 0:1],
                )
                recip = opool.tile([128, 1], f32, tag="rc", name=f"rc_{r}")
                nc.vector.reciprocal(recip[:], sumexp[:])
                nc.vector.tensor_scalar_mul(
                    ot[:, NCLS : 2 * NCLS], ot[:, NCLS : 2 * NCLS], recip[:, 0:1]
                )
                nc.sync.dma_start(out_d[r * 128 : (r + 1) * 128, :], ot[:])

    nc.compile()
    return nc


def _pack_ktiles(a, ncols):
    """[KT*128, ncols] -> [128, KT*ncols] with k-tiles side by side."""
    kt = a.shape[0] // 128
    return np.ascontiguousarray(
        a.reshape(kt, 128, ncols).transpose(1, 0, 2).reshape(128, kt * ncols)
    )


def _prep_inputs(inputs, mm_dtype_name=MM_DTYPE):
    """Host-side reshape/fold/cast. Returns per-core input maps."""
    _, np_mm = _DT_MAP[mm_dtype_name]
    f32 = np.float32

    x = np.asarray(inputs["x"], f32).reshape(N_FULL, K1)
    W1 = np.asarray(inputs["w1"], f32).reshape(K1, H)
    W2 = np.asarray(inputs["w2"], f32)
    wc, bc = np.asarray(inputs["wc"], f32), np.asarray(inputs["bc"], f32)
    wo, bo = np.asarray(inputs["wo"], f32), np.asarray(inputs["bo"], f32)

    s1 = np.asarray(inputs["g1"] / np.sqrt(inputs["v1"] + EPS), f32)
    b1f = np.asarray((inputs["b1"] - inputs["m1"]) * s1 + inputs["be1"], f32)
    s2 = np.asarray(inputs["g2"] / np.sqrt(inputs["v2"] + EPS), f32)
    b2f = np.asarray((inputs["b2"] - inputs["m2"]) * s2 + inputs["be2"], f32)
    # sb[p, m*4+j] = (s1,b1,s2,b2)[j] for channel m*128+p
    sb = np.ascontiguousarray(
        np.stack([s1, b1f, s2, b2f], axis=1).reshape(KT2, 128, 4).transpose(1, 0, 2)
    ).reshape(128, KT2 * 4)

    wcoP = _pack_ktiles(np.concatenate([wc, wo], axis=1), NCO).astype(np_mm)
    bco = np.concatenate([bc, bo]).reshape(1, NCO).astype(np_mm)
    w1P = _pack_ktiles(W1, H).astype(np_mm)
    w2P = _pack_ktiles(W2, H).astype(np_mm)

    in_maps = []
    for c in range(NCORES):
        # xT k-tiles packed along free dim: xp[p, k*NS+r] = x[c*NS+r, k*128+p]
        xs = x[c * NS : (c + 1) * NS]  # [512, 12544]
        xp = np.ascontiguousarray(
            xs.reshape(NS, KT1, 128).transpose(2, 1, 0).reshape(128, KT1 * NS)
        ).astype(np_mm)
        in_maps.append(
            {
                "xp": xp,
                "w1p": w1P,
                "w2p": w2P,
                "wcop": wcoP,
                "bco": bco,
                "sb": sb,
            }
        )
    return in_maps


def _gather(results):
    out = np.concatenate([results[c]["out"] for c in range(NCORES)], axis=0)
    logits = np.ascontiguousarray(out[:, 0:NCLS], dtype=np.float32)
    probs = np.ascontiguousarray(out[:, NCLS : 2 * NCLS], dtype=np.float32)
    offs = np.ascontiguousarray(out[:, 2 * NCLS : NOUT], dtype=np.float32)
    return logits, probs, offs.reshape(N_FULL, NCLS, 4)


def run(inputs, trace=False):
    """Run on 8 cores. Returns ((logits, probs, offset), BassKernelResults)."""
    global _PROG
    if trace:
        _install_ntff_hook()
    if _PROG is None:
        _PROG = _build_program()
    in_maps = _prep_inputs(inputs)
    res = run_bass_kernel_spmd(_PROG, in_maps, list(range(NCORES)), trace=trace)
    return _gather(res.results), res


def kernel(**inputs):
    out, _ = run(inputs, trace=False)
    return out


# revision 8
# speedup vs baseline: 1.0809x; 1.0188x over previous
"""Trainium2 Bass kernel for a detection-head MLP (conv7x7-as-matmul ->
dense -> {classifier softmax, box offsets}), data-parallel over the ROI
axis N on 8 NeuronCores.

Self-contained: hardcodes shapes from the problem spec.
  x  [4096,7,7,256] -> flatten [4096,12544]
  y1 = relu(bn(x @ W1))          [4096,1024]
  y2 = relu(bn(y1 @ W2))         [4096,1024]
  logit = y2 @ wc + bc           [4096,81]
  prob  = softmax(logit)
  off   = (y2 @ wo + bo)         [4096,81,4]

Per-core layout (512 ROIs/core): activations kept transposed
[channels(partition), rows(free)] so every layer is a chain of
128x128-stationary x [128,512]-moving matmuls accumulating in PSUM.
Final layer flips: stationary = y2T tile (k x rows), moving = [wc|wo]
(k x 405), giving row-major [128 rows, 405] outputs; softmax runs along
the free dim. BN+bias folded into a per-channel affine applied during
PSUM->SBUF evacuation on the scalar engine.

All streamed inputs are host-packed so each DMA moves multiple k-tiles
with >=4KB contiguous per partition (small-packet DMA overhead dominated
the v1 profile). Group sizes ramp 1,2,4,7... so the first matmul starts
~1us after launch while steady-state transfers stay large.
"""

import sys
import types

sys.path.insert(0, "/opt/trn_rl_repo")

import numpy as np
import ml_dtypes
from contextlib import ExitStack

import concourse.bass as bass
import concourse.tile as tile
from concourse import bacc, mybir
from concourse.bass_utils import run_bass_kernel_spmd

# ---- problem constants ----
N_FULL = 4096
K1 = 7 * 7 * 256  # 12544
H = 1024
NCLS = 81
NOFF = NCLS * 4  # 324
NCO = NCLS + NOFF  # 405
NOUT = NCLS + NCO  # 486 = logits | probs | offsets packed
EPS = 1e-3
NCORES = 8
NS = N_FULL // NCORES  # 512 rows per core
KT1 = K1 // 128  # 98
KT2 = H // 128  # 8
RT = NS // 128  # 4 row-tiles per core

# k-tile group sizes for the streamed layer-1 operands (sum == KT1).
# Ramp-up start so the PE can begin within ~1us of launch.
GROUPS = [1, 1, 2, 3] + [7] * 13
assert sum(GROUPS) == KT1

# dummy matmuls issued before the first data arrives: ~3.4us of PE activity
# flips the HAM clock gate to 2.4GHz so the real stream starts warm.
WARMUP_MMS = 8

# matmul operand dtype: bfloat16 (fast, ~5e-3 rel err) or float32r
# (TF32-like, ~5e-4 rel err, 2x DMA bytes). Accumulation is fp32 either way.
MM_DTYPE = "bfloat16"

_DT_MAP = {
    "bfloat16": (mybir.dt.bfloat16, ml_dtypes.bfloat16),
    "float32r": (mybir.dt.float32r, np.float32),
}


def _install_ntff_hook():
    """Best-effort registration of the axon NTFF profile hook so callers
    can request trace=True. No-op if the plumbing is unavailable."""
    try:
        import antenv

        if "antenv.axon_hooks" not in sys.modules:
            hooks = types.ModuleType("antenv.axon_hooks")
            hooks._hook = None
            hooks.set_axon_ntff_profile_hook = lambda h: setattr(hooks, "_hook", h)
            hooks.get_axon_ntff_profile_hook = lambda: hooks._hook
            sys.modules["antenv.axon_hooks"] = hooks
            antenv.axon_hooks = hooks
            from trn_agent_boot.trn_boot import _ntff_profile_via_ctypes

            hooks.set_axon_ntff_profile_hook(
                _ntff_profile_via_ctypes("/opt/axon/libaxon_pjrt.so")
            )
    except Exception:
        pass


_PROG = None


def _build_program(mm_dtype_name=MM_DTYPE):
    dt_mm, _ = _DT_MAP[mm_dtype_name]
    f32 = mybir.dt.float32

    nc = bacc.Bacc("TRN2", target_bir_lowering=False, debug=False)

    # xp[g]: k-tiles of xT packed along free dim    [128, gsz*512]
    # w1p[g]: k-tiles of W1 packed along free dim   [128, gsz*1024]
    xp_d = nc.dram_tensor("xp", [128, KT1 * NS], dt_mm, kind="ExternalInput").ap()
    w1_d = nc.dram_tensor("w1p", [128, KT1 * H], dt_mm, kind="ExternalInput").ap()
    w2_d = nc.dram_tensor("w2p", [128, KT2 * H], dt_mm, kind="ExternalInput").ap()
    wco_d = nc.dram_tensor("wcop", [128, KT2 * NCO], dt_mm, kind="ExternalInput").ap()
    bco_d = nc.dram_tensor("bco", [1, NCO], dt_mm, kind="ExternalInput").ap()
    # per-channel affine consts: [128, m*4 + (s1,b1,s2,b2)]
    sb_d = nc.dram_tensor("sb", [128, KT2 * 4], f32, kind="ExternalInput").ap()

    out_d = nc.dram_tensor("out", [NS, NOUT], f32, kind="ExternalOutput").ap()

    with tile.TileContext(nc) as tc:
        with ExitStack() as ctx:
            const = ctx.enter_context(tc.tile_pool(name="const", bufs=1))
            xpool = ctx.enter_context(tc.tile_pool(name="x", bufs=3))
            w1pool = ctx.enter_context(tc.tile_pool(name="w1", bufs=3))
            ypool = ctx.enter_context(tc.tile_pool(name="y", bufs=1))
            opool = ctx.enter_context(tc.tile_pool(name="o", bufs=2))
            pspool = ctx.enter_context(tc.tile_pool(name="ps", bufs=1, space="PSUM"))

            # PE warmup: garbage matmuls on a zeroed scratch tile flip the HAM
            # clock gate to full rate while the first real DMAs are in flight.
            scratch = const.tile([128, NS], dt_mm, tag="scratch")
            nc.gpsimd.memset(scratch[:], 0.0)
            ps_w = pspool.tile([128, NS], f32, tag="ps_0", name="ps_warm")
            for _ in range(WARMUP_MMS):
                nc.tensor.matmul(
                    ps_w[:], scratch[:, 0:128], scratch[:], start=True, stop=True
                )

            # stream the first two layer-1 groups before anything else so the
            # sync-engine DMA queue delivers them with minimum latency
            first_tiles = []
            k0 = 0
            for g, gsz in enumerate(GROUPS[:2]):
                xt = xpool.tile([128, gsz * NS], dt_mm, tag="x", name=f"xt_{g}")
                nc.sync.dma_start(xt[:], xp_d[:, k0 * NS : (k0 + gsz) * NS])
                w1t = w1pool.tile([128, gsz * H], dt_mm, tag="w1", name=f"w1t_{g}")
                nc.sync.dma_start(w1t[:], w1_d[:, k0 * H : (k0 + gsz) * H])
                first_tiles.append((xt, w1t))
                k0 += gsz

            # resident constants: issued on the gpsimd DMA queue so they do
            # not delay the layer-1 stream on the sync queue
            sb_t = const.tile([128, KT2 * 4], f32, tag="sb")
            nc.gpsimd.dma_start(sb_t[:], sb_d[:])
            wco_sb = const.tile([128, KT2 * NCO], dt_mm, tag="wco")
            nc.gpsimd.dma_start(wco_sb[:], wco_d[:])
            bco_sb = const.tile([1, NCO], dt_mm, tag="bco")
            nc.gpsimd.dma_start(bco_sb[:], bco_d[:])
            ones_sb = const.tile([1, 128], dt_mm, tag="ones")
            nc.vector.memset(ones_sb[:], 1.0)
            w2_sb = const.tile([128, KT2 * H], dt_mm, tag="w2")
            nc.gpsimd.dma_start(w2_sb[:], w2_d[:])

            # ---- phase 1: y1T[m] = relu(affine(sum_k W1[k,m].T @ xT[k])) ----
            ps1 = [
                pspool.tile([128, NS], f32, tag=f"ps_{m}", name=f"ps1_{m}")
                for m in range(KT2)
            ]
            k0 = 0
            for g, gsz in enumerate(GROUPS):
                if g < 2:
                    xt, w1t = first_tiles[g]
                else:
                    xt = xpool.tile([128, gsz * NS], dt_mm, tag="x", name=f"xt_{g}")
                    nc.sync.dma_start(xt[:], xp_d[:, k0 * NS : (k0 + gsz) * NS])
                    w1t = w1pool.tile(
                        [128, gsz * H], dt_mm, tag="w1", name=f"w1t_{g}"
                    )
                    nc.sync.dma_start(w1t[:], w1_d[:, k0 * H : (k0 + gsz) * H])
                for j in range(gsz):
                    k = k0 + j
                    for m in range(KT2):
                        nc.tensor.matmul(
                            ps1[m][:],
                            w1t[:, j * H + m * 128 : j * H + (m + 1) * 128],
                            xt[:, j * NS : (j + 1) * NS],
                            start=(k == 0),
                            stop=(k == KT1 - 1),
                        )
                k0 += gsz
            def evacuate(ym, ps, scale_ap, bias_ap, on_act):
                """ym = relu(ps*scale + bias); ACT in one op, DVE in two.
                Alternating engines halves the serial phase-boundary stall."""
                if on_act:
                    nc.scalar.activation(
                        ym[:],
                        ps[:],
                        mybir.ActivationFunctionType.Relu,
                        bias=bias_ap,
                        scale=scale_ap,
                    )
                else:
                    nc.vector.tensor_scalar(
                        ym[:],
                        ps[:],
                        scale_ap,
                        bias_ap,
                        mybir.AluOpType.mult,
                        mybir.AluOpType.add,
                    )
                    nc.vector.tensor_scalar_max(ym[:], ym[:], 0.0)

            y1 = []
            for m in range(KT2):
                ym = ypool.tile([128, NS], dt_mm, tag=f"y1_{m}", name=f"y1t_{m}")
                evacuate(
                    ym,
                    ps1[m],
                    sb_t[:, m * 4 : m * 4 + 1],
                    sb_t[:, m * 4 + 1 : m * 4 + 2],
                    on_act=(m % 2 == 0),
                )
                y1.append(ym)

            # ---- phase 2: y2T[m] = relu(affine(sum_k W2[k,m].T @ y1T[k])) ----
            ps2 = [
                pspool.tile([128, NS], f32, tag=f"ps_{m}", name=f"ps2_{m}")
                for m in range(KT2)
            ]
            for k in range(KT2):
                for m in range(KT2):
                    nc.tensor.matmul(
                        ps2[m][:],
                        w2_sb[:, k * H + m * 128 : k * H + (m + 1) * 128],
                        y1[k][:],
                        start=(k == 0),
                        stop=(k == KT2 - 1),
                    )
            y2 = []
            for m in range(KT2):
                ym = ypool.tile([128, NS], dt_mm, tag=f"y2_{m}", name=f"y2t_{m}")
                evacuate(
                    ym,
                    ps2[m],
                    sb_t[:, m * 4 + 2 : m * 4 + 3],
                    sb_t[:, m * 4 + 3 : m * 4 + 4],
                    on_act=(m % 2 == 0),
                )
                y2.append(ym)

            # ---- phase 3: per 128-row tile, [rows, 405] = y2T.T @ [wc|wo] ----
            # out columns: [0:81) logits, [81:162) softmax probs, [162:486) offsets
            for r in range(RT):
                ps3 = pspool.tile([128, NCO], f32, tag=f"ps_{r}", name=f"ps3_{r}")
                # bias row: ones[1,128].T @ bco[1,405] broadcasts bco to all rows
                nc.tensor.matmul(ps3[:], ones_sb[:], bco_sb[:], start=True, stop=False)
                for k in range(KT2):
                    nc.tensor.matmul(
                        ps3[:],
                        y2[k][:, r * 128 : (r + 1) * 128],
                        wco_sb[:, k * NCO : (k + 1) * NCO],
                        start=False,
                        stop=(k == KT2 - 1),
                    )
                # emission order keeps the probs critical path (reduce -> exp
                # -> recip -> mul) hot while the bulk copies fill engine gaps
                ot = opool.tile([128, NOUT], f32, tag="ot", name=f"ot_{r}")
                negmax = opool.tile([128, 1], f32, tag="nm", name=f"nm_{r}")
                nc.vector.tensor_reduce(
                    negmax[:],
                    ps3[:, 0:NCLS],
                    mybir.AxisListType.X,
                    mybir.AluOpType.max,
                    negate=True,
                )
                sumexp = opool.tile([128, 1], f32, tag="se", name=f"se_{r}")
                nc.scalar.activation(
                    ot[:, NCLS : 2 * NCLS],
                    ps3[:, 0:NCLS],
                    mybir.ActivationFunctionType.Exp,
                    bias=negmax[:, 0:1],
                    scale=1.0,
                    accum_out=sumexp[:, 0:1],
                )
                nc.vector.tensor_copy(ot[:, 2 * NCLS : NOUT], ps3[:, NCLS:NCO])
                recip = opool.tile([128, 1], f32, tag="rc", name=f"rc_{r}")
                nc.vector.reciprocal(recip[:], sumexp[:])
                nc.vector.tensor_scalar_mul(
                    ot[:, NCLS : 2 * NCLS], ot[:, NCLS : 2 * NCLS], recip[:, 0:1]
                )
                nc.scalar.copy(ot[:, 0:NCLS], ps3[:, 0:NCLS])
                nc.sync.dma_start(out_d[r * 128 : (r + 1) * 128, :], ot[:])

    nc.compile()
    return nc


def _pack_ktiles(a, ncols):
    """[KT*128, ncols] -> [128, KT*ncols] with k-tiles side by side."""
    kt = a.shape[0] // 128
    return np.ascontiguousarray(
        a.reshape(kt, 128, ncols).transpose(1, 0, 2).reshape(128, kt * ncols)
    )


def _prep_inputs(inputs, mm_dtype_name=MM_DTYPE):
    """Host-side reshape/fold/cast. Returns per-core input maps."""
    _, np_mm = _DT_MAP[mm_dtype_name]
    f32 = np.float32

    x = np.asarray(inputs["x"], f32).reshape(N_FULL, K1)
    W1 = np.asarray(inputs["w1"], f32).reshape(K1, H)
    W2 = np.asarray(inputs["w2"], f32)
    wc, bc = np.asarray(inputs["wc"], f32), np.asarray(inputs["bc"], f32)
    wo, bo = np.asarray(inputs["wo"], f32), np.asarray(inputs["bo"], f32)

    s1 = np.asarray(inputs["g1"] / np.sqrt(inputs["v1"] + EPS), f32)
    b1f = np.asarray((inputs["b1"] - inputs["m1"]) * s1 + inputs["be1"], f32)
    s2 = np.asarray(inputs["g2"] / np.sqrt(inputs["v2"] + EPS), f32)
    b2f = np.asarray((inputs["b2"] - inputs["m2"]) * s2 + inputs["be2"], f32)
    # sb[p, m*4+j] = (s1,b1,s2,b2)[j] for channel m*128+p
    sb = np.ascontiguousarray(
        np.stack([s1, b1f, s2, b2f], axis=1).reshape(KT2, 128, 4).transpose(1, 0, 2)
    ).reshape(128, KT2 * 4)

    wcoP = _pack_ktiles(np.concatenate([wc, wo], axis=1), NCO).astype(np_mm)
    bco = np.concatenate([bc, bo]).reshape(1, NCO).astype(np_mm)
    w1P = _pack_ktiles(W1, H).astype(np_mm)
    w2P = _pack_ktiles(W2, H).astype(np_mm)

    in_maps = []
    for c in range(NCORES):
        # xT k-tiles packed along free dim: xp[p, k*NS+r] = x[c*NS+r, k*128+p]
        xs = x[c * NS : (c + 1) * NS]  # [512, 12544]
        xp = np.ascontiguousarray(
            xs.reshape(NS, KT1, 128).transpose(2, 1, 0).reshape(128, KT1 * NS)
        ).astype(np_mm)
        in_maps.append(
            {
                "xp": xp,
                "w1p": w1P,
                "w2p": w2P,
                "wcop": wcoP,
                "bco": bco,
                "sb": sb,
            }
        )
    return in_maps


def _gather(results):
    out = np.concatenate([results[c]["out"] for c in range(NCORES)], axis=0)
    logits = np.ascontiguousarray(out[:, 0:NCLS], dtype=np.float32)
    probs = np.ascontiguousarray(out[:, NCLS : 2 * NCLS], dtype=np.float32)
    offs = np.ascontiguousarray(out[:, 2 * NCLS : NOUT], dtype=np.float32)
    return logits, probs, offs.reshape(N_FULL, NCLS, 4)


def run(inputs, trace=False):
    """Run on 8 cores. Returns ((logits, probs, offset), BassKernelResults)."""
    global _PROG
    if trace:
        _install_ntff_hook()
    if _PROG is None:
        _PROG = _build_program()
    in_maps = _prep_inputs(inputs)
    res = run_bass_kernel_spmd(_PROG, in_maps, list(range(NCORES)), trace=trace)
    return _gather(res.results), res


def kernel(**inputs):
    out, _ = run(inputs, trace=False)
    return out


# revision 10
# speedup vs baseline: 1.0971x; 1.0149x over previous
"""Trainium2 Bass kernel for a detection-head MLP (conv7x7-as-matmul ->
dense -> {classifier softmax, box offsets}), data-parallel over the ROI
axis N on 8 NeuronCores.

Self-contained: hardcodes shapes from the problem spec.
  x  [4096,7,7,256] -> flatten [4096,12544]
  y1 = relu(bn(x @ W1))          [4096,1024]
  y2 = relu(bn(y1 @ W2))         [4096,1024]
  logit = y2 @ wc + bc           [4096,81]
  prob  = softmax(logit)
  off   = (y2 @ wo + bo)         [4096,81,4]

Per-core layout (512 ROIs/core): activations kept transposed
[channels(partition), rows(free)] so every layer is a chain of
128x128-stationary x [128,512]-moving matmuls accumulating in PSUM.
Final layer flips: stationary = y2T tile (k x rows), moving = [wc|wo]
(k x 405), giving row-major [128 rows, 405] outputs; softmax runs along
the free dim. BN+bias folded into a per-channel affine applied during
PSUM->SBUF evacuation on the scalar engine.

All streamed inputs are host-packed so each DMA moves multiple k-tiles
with >=4KB contiguous per partition (small-packet DMA overhead dominated
the v1 profile). Group sizes ramp 1,2,4,7... so the first matmul starts
~1us after launch while steady-state transfers stay large.
"""

import sys
import types

sys.path.insert(0, "/opt/trn_rl_repo")

import numpy as np
import ml_dtypes
from contextlib import ExitStack

import concourse.bass as bass
import concourse.tile as tile
from concourse import bacc, mybir
from concourse.bass_utils import run_bass_kernel_spmd

# ---- problem constants ----
N_FULL = 4096
K1 = 7 * 7 * 256  # 12544
H = 1024
NCLS = 81
NOFF = NCLS * 4  # 324
NCO = NCLS + NOFF  # 405
NOUT = NCLS + NCO  # 486 = logits | probs | offsets packed
EPS = 1e-3
NCORES = 8
NS = N_FULL // NCORES  # 512 rows per core
KT1 = K1 // 128  # 98
KT2 = H // 128  # 8
RT = NS // 128  # 4 row-tiles per core

# k-tile group sizes for the streamed layer-1 operands (sum == KT1).
# Ramp-up start so the PE can begin within ~1us of launch.
GROUPS = [1, 1, 2, 3] + [7] * 13
assert sum(GROUPS) == KT1

# dummy matmuls issued before the first data arrives: ~3.4us of PE activity
# flips the HAM clock gate to 2.4GHz so the real stream starts warm.
WARMUP_MMS = 8

# matmul operand dtype: bfloat16 (fast, ~5e-3 rel err) or float32r
# (TF32-like, ~5e-4 rel err, 2x DMA bytes). Accumulation is fp32 either way.
MM_DTYPE = "bfloat16"

_DT_MAP = {
    "bfloat16": (mybir.dt.bfloat16, ml_dtypes.bfloat16),
    "float32r": (mybir.dt.float32r, np.float32),
}


def _install_ntff_hook():
    """Best-effort registration of the axon NTFF profile hook so callers
    can request trace=True. No-op if the plumbing is unavailable."""
    try:
        import antenv

        if "antenv.axon_hooks" not in sys.modules:
            hooks = types.ModuleType("antenv.axon_hooks")
            hooks._hook = None
            hooks.set_axon_ntff_profile_hook = lambda h: setattr(hooks, "_hook", h)
            hooks.get_axon_ntff_profile_hook = lambda: hooks._hook
            sys.modules["antenv.axon_hooks"] = hooks
            antenv.axon_hooks = hooks
            from trn_agent_boot.trn_boot import _ntff_profile_via_ctypes

            hooks.set_axon_ntff_profile_hook(
                _ntff_profile_via_ctypes("/opt/axon/libaxon_pjrt.so")
            )
    except Exception:
        pass


_PROG = None


def _build_program(mm_dtype_name=MM_DTYPE):
    dt_mm, _ = _DT_MAP[mm_dtype_name]
    f32 = mybir.dt.float32

    nc = bacc.Bacc("TRN2", target_bir_lowering=False, debug=False)

    # xp[g]: k-tiles of xT packed along free dim    [128, gsz*512]
    # w1p[g]: k-tiles of W1 packed along free dim   [128, gsz*1024]
    xp_d = nc.dram_tensor("xp", [128, KT1 * NS], dt_mm, kind="ExternalInput").ap()
    w1_d = nc.dram_tensor("w1p", [128, KT1 * H], dt_mm, kind="ExternalInput").ap()
    w2_d = nc.dram_tensor("w2p", [128, KT2 * H], dt_mm, kind="ExternalInput").ap()
    wco_d = nc.dram_tensor("wcop", [128, KT2 * NCO], dt_mm, kind="ExternalInput").ap()
    bco_d = nc.dram_tensor("bco", [1, NCO], dt_mm, kind="ExternalInput").ap()
    # per-channel affine consts: [128, m*4 + (s1,b1,s2,b2)]
    sb_d = nc.dram_tensor("sb", [128, KT2 * 4], f32, kind="ExternalInput").ap()

    out_d = nc.dram_tensor("out", [NS, NOUT], f32, kind="ExternalOutput").ap()

    with tile.TileContext(nc) as tc:
        with ExitStack() as ctx:
            const = ctx.enter_context(tc.tile_pool(name="const", bufs=1))
            xpool = ctx.enter_context(tc.tile_pool(name="x", bufs=4))
            w1pool = ctx.enter_context(tc.tile_pool(name="w1", bufs=4))
            ypool = ctx.enter_context(tc.tile_pool(name="y", bufs=1))
            opool = ctx.enter_context(tc.tile_pool(name="o", bufs=2))
            pspool = ctx.enter_context(tc.tile_pool(name="ps", bufs=1, space="PSUM"))

            # PE warmup: garbage matmuls on a zeroed scratch tile flip the HAM
            # clock gate to full rate while the first real DMAs are in flight.
            scratch = const.tile([128, NS], dt_mm, tag="scratch")
            nc.gpsimd.memset(scratch[:], 0.0)
            ps_w = pspool.tile([128, NS], f32, tag="ps_0", name="ps_warm")
            for _ in range(WARMUP_MMS):
                nc.tensor.matmul(
                    ps_w[:], scratch[:, 0:128], scratch[:], start=True, stop=True
                )

            ones_sb = const.tile([1, 128], dt_mm, tag="ones")
            nc.vector.memset(ones_sb[:], 1.0)
            # resident-constant tiles; their DMAs are emitted AFTER the
            # layer-1 stream so the sync queue delivers the stream first
            # (consts are not consumed until >150us into the kernel)
            sb_t = const.tile([128, KT2 * 4], f32, tag="sb")
            wco_sb = const.tile([128, KT2 * NCO], dt_mm, tag="wco")
            bco_sb = const.tile([1, NCO], dt_mm, tag="bco")
            w2_sb = const.tile([128, KT2 * H], dt_mm, tag="w2")

            # ---- phase 1: y1T[m] = relu(affine(sum_k W1[k,m].T @ xT[k])) ----
            ps1 = [
                pspool.tile([128, NS], f32, tag=f"ps_{m}", name=f"ps1_{m}")
                for m in range(KT2)
            ]
            k0 = 0
            for g, gsz in enumerate(GROUPS):
                xt = xpool.tile([128, gsz * NS], dt_mm, tag="x", name=f"xt_{g}")
                nc.sync.dma_start(xt[:], xp_d[:, k0 * NS : (k0 + gsz) * NS])
                w1t = w1pool.tile([128, gsz * H], dt_mm, tag="w1", name=f"w1t_{g}")
                nc.sync.dma_start(w1t[:], w1_d[:, k0 * H : (k0 + gsz) * H])
                if g == len(GROUPS) - 1:
                    # queue the constants behind the full layer-1 stream
                    nc.sync.dma_start(sb_t[:], sb_d[:])
                    nc.sync.dma_start(wco_sb[:], wco_d[:])
                    nc.sync.dma_start(bco_sb[:], bco_d[:])
                    nc.sync.dma_start(w2_sb[:], w2_d[:])
                for j in range(gsz):
                    k = k0 + j
                    for m in range(KT2):
                        nc.tensor.matmul(
                            ps1[m][:],
                            w1t[:, j * H + m * 128 : j * H + (m + 1) * 128],
                            xt[:, j * NS : (j + 1) * NS],
                            start=(k == 0),
                            stop=(k == KT1 - 1),
                        )
                k0 += gsz
            def evacuate(ym, ps, scale_ap, bias_ap, on_act):
                """ym = relu(ps*scale + bias); ACT in one op, DVE in two.
                Alternating engines halves the serial phase-boundary stall."""
                if on_act:
                    nc.scalar.activation(
                        ym[:],
                        ps[:],
                        mybir.ActivationFunctionType.Relu,
                        bias=bias_ap,
                        scale=scale_ap,
                    )
                else:
                    nc.vector.tensor_scalar(
                        ym[:],
                        ps[:],
                        scale_ap,
                        bias_ap,
                        mybir.AluOpType.mult,
                        mybir.AluOpType.add,
                    )
                    nc.vector.tensor_scalar_max(ym[:], ym[:], 0.0)

            y1 = []
            for m in range(KT2):
                ym = ypool.tile([128, NS], dt_mm, tag=f"y1_{m}", name=f"y1t_{m}")
                evacuate(
                    ym,
                    ps1[m],
                    sb_t[:, m * 4 : m * 4 + 1],
                    sb_t[:, m * 4 + 1 : m * 4 + 2],
                    on_act=(m % 2 == 0),
                )
                y1.append(ym)

            # ---- phase 2: y2T[m] = relu(affine(sum_k W2[k,m].T @ y1T[k])) ----
            ps2 = [
                pspool.tile([128, NS], f32, tag=f"ps_{m}", name=f"ps2_{m}")
                for m in range(KT2)
            ]
            for k in range(KT2):
                for m in range(KT2):
                    nc.tensor.matmul(
                        ps2[m][:],
                        w2_sb[:, k * H + m * 128 : k * H + (m + 1) * 128],
                        y1[k][:],
                        start=(k == 0),
                        stop=(k == KT2 - 1),
                    )
            y2 = []
            for m in range(KT2):
                ym = ypool.tile([128, NS], dt_mm, tag=f"y2_{m}", name=f"y2t_{m}")
                evacuate(
                    ym,
                    ps2[m],
                    sb_t[:, m * 4 + 2 : m * 4 + 3],
                    sb_t[:, m * 4 + 3 : m * 4 + 4],
                    on_act=(m % 2 == 0),
                )
                y2.append(ym)

            # ---- phase 3: per 128-row tile, [rows, 405] = y2T.T @ [wc|wo] ----
            # out columns: [0:81) logits, [81:162) softmax probs, [162:486) offsets
            for r in range(RT):
                ps3 = pspool.tile([128, NCO], f32, tag=f"ps_{r}", name=f"ps3_{r}")
                # bias row: ones[1,128].T @ bco[1,405] broadcasts bco to all rows
                nc.tensor.matmul(ps3[:], ones_sb[:], bco_sb[:], start=True, stop=False)
                for k in range(KT2):
                    nc.tensor.matmul(
                        ps3[:],
                        y2[k][:, r * 128 : (r + 1) * 128],
                        wco_sb[:, k * NCO : (k + 1) * NCO],
                        start=False,
                        stop=(k == KT2 - 1),
                    )
                # emission order keeps the probs critical path (reduce -> exp
                # -> recip -> mul) hot while the bulk copies fill engine gaps
                ot = opool.tile([128, NOUT], f32, tag="ot", name=f"ot_{r}")
                negmax = opool.tile([128, 1], f32, tag="nm", name=f"nm_{r}")
                nc.vector.tensor_reduce(
                    negmax[:],
                    ps3[:, 0:NCLS],
                    mybir.AxisListType.X,
                    mybir.AluOpType.max,
                    negate=True,
                )
                sumexp = opool.tile([128, 1], f32, tag="se", name=f"se_{r}")
                nc.scalar.activation(
                    ot[:, NCLS : 2 * NCLS],
                    ps3[:, 0:NCLS],
                    mybir.ActivationFunctionType.Exp,
                    bias=negmax[:, 0:1],
                    scale=1.0,
                    accum_out=sumexp[:, 0:1],
                )
                nc.vector.tensor_copy(ot[:, 2 * NCLS : NOUT], ps3[:, NCLS:NCO])
                recip = opool.tile([128, 1], f32, tag="rc", name=f"rc_{r}")
                nc.vector.reciprocal(recip[:], sumexp[:])
                nc.vector.tensor_scalar_mul(
                    ot[:, NCLS : 2 * NCLS], ot[:, NCLS : 2 * NCLS], recip[:, 0:1]
                )
                nc.scalar.copy(ot[:, 0:NCLS], ps3[:, 0:NCLS])
                nc.sync.dma_start(out_d[r * 128 : (r + 1) * 128, :], ot[:])

    nc.compile()
    return nc


def _pack_ktiles(a, ncols):
    """[KT*128, ncols] -> [128, KT*ncols] with k-tiles side by side."""
    kt = a.shape[0] // 128
    return np.ascontiguousarray(
        a.reshape(kt, 128, ncols).transpose(1, 0, 2).reshape(128, kt * ncols)
    )


def _prep_inputs(inputs, mm_dtype_name=MM_DTYPE):
    """Host-side reshape/fold/cast. Returns per-core input maps."""
    _, np_mm = _DT_MAP[mm_dtype_name]
    f32 = np.float32

    x = np.asarray(inputs["x"], f32).reshape(N_FULL, K1)
    W1 = np.asarray(inputs["w1"], f32).reshape(K1, H)
    W2 = np.asarray(inputs["w2"], f32)
    wc, bc = np.asarray(inputs["wc"], f32), np.asarray(inputs["bc"], f32)
    wo, bo = np.asarray(inputs["wo"], f32), np.asarray(inputs["bo"], f32)

    s1 = np.asarray(inputs["g1"] / np.sqrt(inputs["v1"] + EPS), f32)
    b1f = np.asarray((inputs["b1"] - inputs["m1"]) * s1 + inputs["be1"], f32)
    s2 = np.asarray(inputs["g2"] / np.sqrt(inputs["v2"] + EPS), f32)
    b2f = np.asarray((inputs["b2"] - inputs["m2"]) * s2 + inputs["be2"], f32)
    # sb[p, m*4+j] = (s1,b1,s2,b2)[j] for channel m*128+p
    sb = np.ascontiguousarray(
        np.stack([s1, b1f, s2, b2f], axis=1).reshape(KT2, 128, 4).transpose(1, 0, 2)
    ).reshape(128, KT2 * 4)

    wcoP = _pack_ktiles(np.concatenate([wc, wo], axis=1), NCO).astype(np_mm)
    bco = np.concatenate([bc, bo]).reshape(1, NCO).astype(np_mm)
    w1P = _pack_ktiles(W1, H).astype(np_mm)
    w2P = _pack_ktiles(W2, H).astype(np_mm)

    in_maps = []
    for c in range(NCORES):
        # xT k-tiles packed along free dim: xp[p, k*NS+r] = x[c*NS+r, k*128+p]
        xs = x[c * NS : (c + 1) * NS]  # [512, 12544]
        xp = np.ascontiguousarray(
            xs.reshape(NS, KT1, 128).transpose(2, 1, 0).reshape(128, KT1 * NS)
        ).astype(np_mm)
        in_maps.append(
            {
                "xp": xp,
                "w1p": w1P,
                "w2p": w2P,
                "wcop": wcoP,
                "bco": bco,
                "sb": sb,
            }
        )
    return in_maps


def _gather(results):
    out = np.concatenate([results[c]["out"] for c in range(NCORES)], axis=0)
    logits = np.ascontiguousarray(out[:, 0:NCLS], dtype=np.float32)
    probs = np.ascontiguousarray(out[:, NCLS : 2 * NCLS], dtype=np.float32)
    offs = np.ascontiguousarray(out[:, 2 * NCLS : NOUT], dtype=np.float32)
    return logits, probs, offs.reshape(N_FULL, NCLS, 4)


def run(inputs, trace=False):
    """Run on 8 cores. Returns ((logits, probs, offset), BassKernelResults)."""
    global _PROG
    if trace:
        _install_ntff_hook()
    if _PROG is None:
        _PROG = _build_program()
    in_maps = _prep_inputs(inputs)
    res = run_bass_kernel_spmd(_PROG, in_maps, list(range(NCORES)), trace=trace)
    return _gather(res.results), res


def kernel(**inputs):
    out, _ = run(inputs, trace=False)
    return out


# revision 12
# speedup vs baseline: 1.1296x; 1.0297x over previous
"""Trainium2 Bass kernel for a detection-head MLP (conv7x7-as-matmul ->
dense -> {classifier softmax, box offsets}), data-parallel over the ROI
axis N on 8 NeuronCores.

Self-contained: hardcodes shapes from the problem spec.
  x  [4096,7,7,256] -> flatten [4096,12544]
  y1 = relu(bn(x @ W1))          [4096,1024]
  y2 = relu(bn(y1 @ W2))         [4096,1024]
  logit = y2 @ wc + bc           [4096,81]
  prob  = softmax(logit)
  off   = (y2 @ wo + bo)         [4096,81,4]

Per-core layout (512 ROIs/core): activations kept transposed
[channels(partition), rows(free)] so every layer is a chain of
128x128-stationary x [128,512]-moving matmuls accumulating in PSUM.
Final layer flips: stationary = y2T tile (k x rows), moving = [wc|wo]
(k x 405), giving row-major [128 rows, 405] outputs; softmax runs along
the free dim. BN+bias folded into a per-channel affine applied during
PSUM->SBUF evacuation on the scalar engine.

All streamed inputs are host-packed so each DMA moves multiple k-tiles
with >=4KB contiguous per partition (small-packet DMA overhead dominated
the v1 profile). Group sizes ramp 1,2,4,7... so the first matmul starts
~1us after launch while steady-state transfers stay large.
"""

import sys
import types

sys.path.insert(0, "/opt/trn_rl_repo")

import numpy as np
import ml_dtypes
from contextlib import ExitStack

import concourse.bass as bass
import concourse.tile as tile
from concourse import bacc, mybir
from concourse.bass_utils import run_bass_kernel_spmd

# ---- problem constants ----
N_FULL = 4096
K1 = 7 * 7 * 256  # 12544
H = 1024
NCLS = 81
NOFF = NCLS * 4  # 324
NCO = NCLS + NOFF  # 405
NOUT = NCLS + NCO  # 486 = logits | probs | offsets packed
EPS = 1e-3
NCORES = 8
NS = N_FULL // NCORES  # 512 rows per core
KT1 = K1 // 128  # 98
KT2 = H // 128  # 8
RT = NS // 128  # 4 row-tiles per core

# k-tile group sizes for the streamed layer-1 operands (sum == KT1).
# 2-k-tile groups: 256KB/512KB transfers (2-4KB per partition, full DMA
# throughput) whose readiness lag (~2.1us) stays under the PE's 3.5us
# consumption time, so the stream never underruns.
GROUPS = [2] * 49
assert sum(GROUPS) == KT1

# dummy matmuls issued before the first data arrives: ~3.4us of PE activity
# flips the HAM clock gate to 2.4GHz so the real stream starts warm.
WARMUP_MMS = 8

# matmul operand dtype: bfloat16 (fast, ~5e-3 rel err) or float32r
# (TF32-like, ~5e-4 rel err, 2x DMA bytes). Accumulation is fp32 either way.
MM_DTYPE = "bfloat16"

_DT_MAP = {
    "bfloat16": (mybir.dt.bfloat16, ml_dtypes.bfloat16),
    "float32r": (mybir.dt.float32r, np.float32),
}


def _install_ntff_hook():
    """Best-effort registration of the axon NTFF profile hook so callers
    can request trace=True. No-op if the plumbing is unavailable."""
    try:
        import antenv

        if "antenv.axon_hooks" not in sys.modules:
            hooks = types.ModuleType("antenv.axon_hooks")
            hooks._hook = None
            hooks.set_axon_ntff_profile_hook = lambda h: setattr(hooks, "_hook", h)
            hooks.get_axon_ntff_profile_hook = lambda: hooks._hook
            sys.modules["antenv.axon_hooks"] = hooks
            antenv.axon_hooks = hooks
            from trn_agent_boot.trn_boot import _ntff_profile_via_ctypes

            hooks.set_axon_ntff_profile_hook(
                _ntff_profile_via_ctypes("/opt/axon/libaxon_pjrt.so")
            )
    except Exception:
        pass


_PROG = None


def _build_program(mm_dtype_name=MM_DTYPE):
    dt_mm, _ = _DT_MAP[mm_dtype_name]
    f32 = mybir.dt.float32

    nc = bacc.Bacc("TRN2", target_bir_lowering=False, debug=False)

    # xp[g]: k-tiles of xT packed along free dim    [128, gsz*512]
    # w1p[g]: k-tiles of W1 packed along free dim   [128, gsz*1024]
    xp_d = nc.dram_tensor("xp", [128, KT1 * NS], dt_mm, kind="ExternalInput").ap()
    w1_d = nc.dram_tensor("w1p", [128, KT1 * H], dt_mm, kind="ExternalInput").ap()
    w2_d = nc.dram_tensor("w2p", [128, KT2 * H], dt_mm, kind="ExternalInput").ap()
    wco_d = nc.dram_tensor("wcop", [128, KT2 * NCO], dt_mm, kind="ExternalInput").ap()
    bco_d = nc.dram_tensor("bco", [1, NCO], dt_mm, kind="ExternalInput").ap()
    # per-channel affine consts: [128, m*4 + (s1,b1,s2,b2)]
    sb_d = nc.dram_tensor("sb", [128, KT2 * 4], f32, kind="ExternalInput").ap()

    out_d = nc.dram_tensor("out", [NS, NOUT], f32, kind="ExternalOutput").ap()

    with tile.TileContext(nc) as tc:
        with ExitStack() as ctx:
            const = ctx.enter_context(tc.tile_pool(name="const", bufs=1))
            xpool = ctx.enter_context(tc.tile_pool(name="x", bufs=4))
            w1pool = ctx.enter_context(tc.tile_pool(name="w1", bufs=4))
            ypool = ctx.enter_context(tc.tile_pool(name="y", bufs=1))
            opool = ctx.enter_context(tc.tile_pool(name="o", bufs=2))
            pspool = ctx.enter_context(tc.tile_pool(name="ps", bufs=1, space="PSUM"))

            # PE warmup: garbage matmuls on a zeroed scratch tile flip the HAM
            # clock gate to full rate while the first real DMAs are in flight.
            scratch = const.tile([128, NS], dt_mm, tag="scratch")
            nc.gpsimd.memset(scratch[:], 0.0)
            ps_w = pspool.tile([128, NS], f32, tag="ps_0", name="ps_warm")
            for _ in range(WARMUP_MMS):
                nc.tensor.matmul(
                    ps_w[:], scratch[:, 0:128], scratch[:], start=True, stop=True
                )

            ones_sb = const.tile([1, 128], dt_mm, tag="ones")
            nc.vector.memset(ones_sb[:], 1.0)
            # resident-constant tiles; their DMAs are emitted AFTER the
            # layer-1 stream so the sync queue delivers the stream first
            # (consts are not consumed until >150us into the kernel)
            sb_t = const.tile([128, KT2 * 4], f32, tag="sb")
            wco_sb = const.tile([128, KT2 * NCO], dt_mm, tag="wco")
            bco_sb = const.tile([1, NCO], dt_mm, tag="bco")
            w2_sb = const.tile([128, KT2 * H], dt_mm, tag="w2")

            # ---- phase 1: y1T[m] = relu(affine(sum_k W1[k,m].T @ xT[k])) ----
            ps1 = [
                pspool.tile([128, NS], f32, tag=f"ps_{m}", name=f"ps1_{m}")
                for m in range(KT2)
            ]
            k0 = 0
            for g, gsz in enumerate(GROUPS):
                xt = xpool.tile([128, gsz * NS], dt_mm, tag="x", name=f"xt_{g}")
                nc.sync.dma_start(xt[:], xp_d[:, k0 * NS : (k0 + gsz) * NS])
                w1t = w1pool.tile([128, gsz * H], dt_mm, tag="w1", name=f"w1t_{g}")
                nc.sync.dma_start(w1t[:], w1_d[:, k0 * H : (k0 + gsz) * H])
                if g == len(GROUPS) - 1:
                    # queue the constants behind the full layer-1 stream
                    nc.sync.dma_start(sb_t[:], sb_d[:])
                    nc.sync.dma_start(wco_sb[:], wco_d[:])
                    nc.sync.dma_start(bco_sb[:], bco_d[:])
                    nc.sync.dma_start(w2_sb[:], w2_d[:])
                for j in range(gsz):
                    k = k0 + j
                    for m in range(KT2):
                        nc.tensor.matmul(
                            ps1[m][:],
                            w1t[:, j * H + m * 128 : j * H + (m + 1) * 128],
                            xt[:, j * NS : (j + 1) * NS],
                            start=(k == 0),
                            stop=(k == KT1 - 1),
                        )
                k0 += gsz
            def evacuate(ym, ps, scale_ap, bias_ap, on_act):
                """ym = relu(ps*scale + bias); ACT in one op, DVE in two.
                Alternating engines halves the serial phase-boundary stall."""
                if on_act:
                    nc.scalar.activation(
                        ym[:],
                        ps[:],
                        mybir.ActivationFunctionType.Relu,
                        bias=bias_ap,
                        scale=scale_ap,
                    )
                else:
                    nc.vector.tensor_scalar(
                        ym[:],
                        ps[:],
                        scale_ap,
                        bias_ap,
                        mybir.AluOpType.mult,
                        mybir.AluOpType.add,
                    )
                    nc.vector.tensor_scalar_max(ym[:], ym[:], 0.0)

            y1 = []
            for m in range(KT2):
                ym = ypool.tile([128, NS], dt_mm, tag=f"y1_{m}", name=f"y1t_{m}")
                evacuate(
                    ym,
                    ps1[m],
                    sb_t[:, m * 4 : m * 4 + 1],
                    sb_t[:, m * 4 + 1 : m * 4 + 2],
                    on_act=(m % 2 == 0),
                )
                y1.append(ym)

            # ---- phase 2: y2T[m] = relu(affine(sum_k W2[k,m].T @ y1T[k])) ----
            ps2 = [
                pspool.tile([128, NS], f32, tag=f"ps_{m}", name=f"ps2_{m}")
                for m in range(KT2)
            ]
            for k in range(KT2):
                for m in range(KT2):
                    nc.tensor.matmul(
                        ps2[m][:],
                        w2_sb[:, k * H + m * 128 : k * H + (m + 1) * 128],
                        y1[k][:],
                        start=(k == 0),
                        stop=(k == KT2 - 1),
                    )
            y2 = []
            for m in range(KT2):
                ym = ypool.tile([128, NS], dt_mm, tag=f"y2_{m}", name=f"y2t_{m}")
                evacuate(
                    ym,
                    ps2[m],
                    sb_t[:, m * 4 + 2 : m * 4 + 3],
                    sb_t[:, m * 4 + 3 : m * 4 + 4],
                    on_act=(m % 2 == 0),
                )
                y2.append(ym)

            # ---- phase 3: per 128-row tile, [rows, 405] = y2T.T @ [wc|wo] ----
            # out columns: [0:81) logits, [81:162) softmax probs, [162:486) offsets
            for r in range(RT):
                ps3 = pspool.tile([128, NCO], f32, tag=f"ps_{r}", name=f"ps3_{r}")
                # bias row: ones[1,128].T @ bco[1,405] broadcasts bco to all rows
                nc.tensor.matmul(ps3[:], ones_sb[:], bco_sb[:], start=True, stop=False)
                for k in range(KT2):
                    nc.tensor.matmul(
                        ps3[:],
                        y2[k][:, r * 128 : (r + 1) * 128],
                        wco_sb[:, k * NCO : (k + 1) * NCO],
                        start=False,
                        stop=(k == KT2 - 1),
                    )
                # emission order keeps the probs critical path (exp -> recip
                # -> mul) hot while the bulk copies fill engine gaps. Logits
                # are O(1) (bn-normalized activations x 1/sqrt(H)-scaled
                # weights), far from fp32 exp overflow, so no max-subtraction
                # is needed -- softmax is shift-invariant and the reference
                # only subtracts the max for range safety.
                ot = opool.tile([128, NOUT], f32, tag="ot", name=f"ot_{r}")
                sumexp = opool.tile([128, 1], f32, tag="se", name=f"se_{r}")
                nc.scalar.activation(
                    ot[:, NCLS : 2 * NCLS],
                    ps3[:, 0:NCLS],
                    mybir.ActivationFunctionType.Exp,
                    bias=0.0,
                    scale=1.0,
                    accum_out=sumexp[:, 0:1],
                )
                nc.vector.tensor_copy(ot[:, 2 * NCLS : NOUT], ps3[:, NCLS:NCO])
                recip = opool.tile([128, 1], f32, tag="rc", name=f"rc_{r}")
                nc.vector.reciprocal(recip[:], sumexp[:])
                nc.vector.tensor_scalar_mul(
                    ot[:, NCLS : 2 * NCLS], ot[:, NCLS : 2 * NCLS], recip[:, 0:1]
                )
                nc.scalar.copy(ot[:, 0:NCLS], ps3[:, 0:NCLS])
                nc.sync.dma_start(out_d[r * 128 : (r + 1) * 128, :], ot[:])

    nc.compile()
    return nc


def _pack_ktiles(a, ncols):
    """[KT*128, ncols] -> [128, KT*ncols] with k-tiles side by side."""
    kt = a.shape[0] // 128
    return np.ascontiguousarray(
        a.reshape(kt, 128, ncols).transpose(1, 0, 2).reshape(128, kt * ncols)
    )


def _prep_inputs(inputs, mm_dtype_name=MM_DTYPE):
    """Host-side reshape/fold/cast. Returns per-core input maps."""
    _, np_mm = _DT_MAP[mm_dtype_name]
    f32 = np.float32

    x = np.asarray(inputs["x"], f32).reshape(N_FULL, K1)
    W1 = np.asarray(inputs["w1"], f32).reshape(K1, H)
    W2 = np.asarray(inputs["w2"], f32)
    wc, bc = np.asarray(inputs["wc"], f32), np.asarray(inputs["bc"], f32)
    wo, bo = np.asarray(inputs["wo"], f32), np.asarray(inputs["bo"], f32)

    s1 = np.asarray(inputs["g1"] / np.sqrt(inputs["v1"] + EPS), f32)
    b1f = np.asarray((inputs["b1"] - inputs["m1"]) * s1 + inputs["be1"], f32)
    s2 = np.asarray(inputs["g2"] / np.sqrt(inputs["v2"] + EPS), f32)
    b2f = np.asarray((inputs["b2"] - inputs["m2"]) * s2 + inputs["be2"], f32)
    # sb[p, m*4+j] = (s1,b1,s2,b2)[j] for channel m*128+p
    sb = np.ascontiguousarray(
        np.stack([s1, b1f, s2, b2f], axis=1).reshape(KT2, 128, 4).transpose(1, 0, 2)
    ).reshape(128, KT2 * 4)

    wcoP = _pack_ktiles(np.concatenate([wc, wo], axis=1), NCO).astype(np_mm)
    bco = np.concatenate([bc, bo]).reshape(1, NCO).astype(np_mm)
    w1P = _pack_ktiles(W1, H).astype(np_mm)
    w2P = _pack_ktiles(W2, H).astype(np_mm)

    in_maps = []
    for c in range(NCORES):
        # xT k-tiles packed along free dim: xp[p, k*NS+r] = x[c*NS+r, k*128+p]
        xs = x[c * NS : (c + 1) * NS]  # [512, 12544]
        xp = np.ascontiguousarray(
            xs.reshape(NS, KT1, 128).transpose(2, 1, 0).reshape(128, KT1 * NS)
        ).astype(np_mm)
        in_maps.append(
            {
                "xp": xp,
                "w1p": w1P,
                "w2p": w2P,
                "wcop": wcoP,
                "bco": bco,
                "sb": sb,
            }
        )
    return in_maps


def _gather(results):
    out = np.concatenate([results[c]["out"] for c in range(NCORES)], axis=0)
    logits = np.ascontiguousarray(out[:, 0:NCLS], dtype=np.float32)
    probs = np.ascontiguousarray(out[:, NCLS : 2 * NCLS], dtype=np.float32)
    offs = np.ascontiguousarray(out[:, 2 * NCLS : NOUT], dtype=np.float32)
    return logits, probs, offs.reshape(N_FULL, NCLS, 4)


def run(inputs, trace=False):
    """Run on 8 cores. Returns ((logits, probs, offset), BassKernelResults)."""
    global _PROG
    if trace:
        _install_ntff_hook()
    if _PROG is None:
        _PROG = _build_program()
    in_maps = _prep_inputs(inputs)
    res = run_bass_kernel_spmd(_PROG, in_maps, list(range(NCORES)), trace=trace)
    return _gather(res.results), res


def kernel(**inputs):
    out, _ = run(inputs, trace=False)
    return out
